# revision 8
# baseline (speedup 1.0000x reference)
"""Trainium2 Bass kernel for nn_MultiHeadAttention (dense transformer MHA).

Strategy (8-way tensor parallel over heads):
  - Each of the 8 cores owns 2 heads (128 of the 1024 q/k/v features).
  - Host pre-transposes the activations (query/key/value -> [D, T]) and casts
    to bf16; weights are head-sliced per core (Wo stays full).
  - The reference's RoPE variant uses neg_half = [y1, -y2] (not the usual
    rotate-half), which makes it a purely ELEMENTWISE transform:
        rope(y)[t, f] = y[t, f] * (cos(t*th_f) + sign_f * sin(t*th_f))
    so it is applied as one multiply by a host-precomputed factor C^T.
  - Attention is computed in the transposed layout S^T[s, t] so the P@V
    matmul needs no transposes.  Softmax is the "unsafe" variant (max |logit|
    ~ 10, exp is safe in fp32): exp on the Scalar engine, the denominator is
    obtained by appending a ones-column to V in the U = V'^T @ exp(S^T)
    matmul (row 64 of U accumulates colsum), and normalization is a
    partition-broadcast + elementwise multiply.
  - Re-partition from head-sharding to sequence-sharding with two AllToAll
    collectives (one per T/2 block, the first overlapped with remaining
    compute); each core then projects its own 2x128 output rows through the
    full Wo and returns a [256, 1024] shard which the host concatenates.
"""
import numpy as np
import ml_dtypes

import concourse.bass as bass
import concourse.mybir as mybir
import concourse.tile as tile
from concourse import bacc
from concourse.bass_utils import run_bass_kernel_spmd

# problem constants (hardcoded per contract)
T = 2048
D = 1024
H = 16
DH = 64
ROPE_BASE = 10000

N_CORES = 8
HPC = H // N_CORES          # heads per core = 2
FPC = HPC * DH              # features per core = 128
TC = 512                    # attention t-chunk
NTC = T // TC               # 4
NS = T // 128               # 16 s-tiles
ND = D // 128               # 8 d-tiles
VW = 2 * DH + 2             # 130: v_ext block width per s-tile
ROWS = T // (2 * N_CORES)   # 128 output rows per core per A2A half

bf16 = mybir.dt.bfloat16
f32 = mybir.dt.float32
EXP = mybir.ActivationFunctionType.Exp

_cache = {}


def _build():
    nc = bacc.Bacc("TRN2", target_bir_lowering=False, debug=False,
                   num_devices=N_CORES)

    # ---- I/O -----------------------------------------------------------
    qT = nc.dram_tensor("qT", [D, T], bf16, kind="ExternalInput").ap()
    kT = nc.dram_tensor("kT", [D, T], bf16, kind="ExternalInput").ap()
    vT = nc.dram_tensor("vT", [D, T], bf16, kind="ExternalInput").ap()
    wq = nc.dram_tensor("wq", [D, FPC], bf16, kind="ExternalInput").ap()
    wk = nc.dram_tensor("wk", [D, FPC], bf16, kind="ExternalInput").ap()
    wv = nc.dram_tensor("wv", [D, FPC], bf16, kind="ExternalInput").ap()
    wo = nc.dram_tensor("wo", [D, D], bf16, kind="ExternalInput").ap()
    bq = nc.dram_tensor("bq", [1, FPC], bf16, kind="ExternalInput").ap()
    bk = nc.dram_tensor("bk", [1, FPC], bf16, kind="ExternalInput").ap()
    bv = nc.dram_tensor("bv", [1, FPC], bf16, kind="ExternalInput").ap()
    bo = nc.dram_tensor("bo", [1, D], bf16, kind="ExternalInput").ap()
    ropeC = nc.dram_tensor("ropeC", [FPC, T], f32, kind="ExternalInput").ap()
    out1 = nc.dram_tensor("out1", [ROWS, D], f32, kind="ExternalOutput").ap()
    out2 = nc.dram_tensor("out2", [ROWS, D], f32, kind="ExternalOutput").ap()

    with tile.TileContext(nc) as tc:
        with (
            tc.tile_pool(name="win", bufs=1) as win,        # weights/consts
            tc.tile_pool(name="xin", bufs=24) as xin,       # input stream
            tc.tile_pool(name="qk", bufs=1) as qkpool,      # q^T / k^T
            tc.tile_pool(name="vx", bufs=1) as vxpool,      # v_ext
            tc.tile_pool(name="ex", bufs=3) as expool,      # exp(S^T)
            tc.tile_pool(name="at", bufs=1) as atpool,      # attn^T halves
            tc.tile_pool(name="nrm", bufs=4) as nrmpool,    # u_sb / Rbc
            tc.tile_pool(name="opr", bufs=2) as oprpool,    # out-proj tiles
            tc.tile_pool(name="pp", bufs=2, space="PSUM") as pproj,
            tc.tile_pool(name="ps", bufs=2, space="PSUM") as pS,
            tc.tile_pool(name="pu", bufs=2, space="PSUM") as pU,
            tc.tile_pool(name="dram", bufs=1, space="DRAM") as dram,
        ):
            # ---- constants / weights ----------------------------------
            wq_sb = win.tile([128, ND * FPC], bf16, tag="wq")
            wk_sb = win.tile([128, ND * FPC], bf16, tag="wk")
            wv_sb = win.tile([128, ND * FPC], bf16, tag="wv")
            for w_sb, w in ((wq_sb, wq), (wk_sb, wk), (wv_sb, wv)):
                nc.sync.dma_start(
                    out=w_sb[:].rearrange("p (d m) -> p d m", d=ND),
                    in_=w.rearrange("(d p) m -> p d m", p=128))
            bq_sb = win.tile([1, FPC], bf16, tag="bq")
            bk_sb = win.tile([1, FPC], bf16, tag="bk")
            bv_sb = win.tile([1, FPC], bf16, tag="bv")
            bo_sb = win.tile([1, D], bf16, tag="bo")
            for b_sb, b in ((bq_sb, bq), (bk_sb, bk), (bv_sb, bv), (bo_sb, bo)):
                nc.sync.dma_start(out=b_sb[:], in_=b)
            rope_sb = win.tile([FPC, T], f32, tag="rope")
            nc.sync.dma_start(out=rope_sb[:], in_=ropeC)
            ones_sb = win.tile([1, T], bf16, tag="ones")
            nc.gpsimd.memset(ones_sb[:], 1.0)

            # ---- stream inputs ----------------------------------------
            qin = [xin.tile([128, T], bf16, tag="xin", name=f"qin{d}")
                   for d in range(ND)]
            kin = [xin.tile([128, T], bf16, tag="xin", name=f"kin{d}")
                   for d in range(ND)]
            vin = [xin.tile([128, T], bf16, tag="xin", name=f"vin{d}")
                   for d in range(ND)]
            for d in range(ND):
                nc.sync.dma_start(out=qin[d][:], in_=qT[128 * d:128 * (d + 1), :])
            for d in range(ND):
                nc.sync.dma_start(out=kin[d][:], in_=kT[128 * d:128 * (d + 1), :])
            for d in range(ND):
                nc.sync.dma_start(out=vin[d][:], in_=vT[128 * d:128 * (d + 1), :])

            # ---- projections ------------------------------------------
            qt_sb = qkpool.tile([128, T], bf16, tag="qt")
            kt_sb = qkpool.tile([128, T], bf16, tag="kt")

            # q^T / k^T: out [FPC, T] = W.T @ xT, rope applied on eviction
            for x_sb, w_sb, b_sb, xtiles in (
                (qt_sb, wq_sb, bq_sb, qin),
                (kt_sb, wk_sb, bk_sb, kin),
            ):
                for tc_i in range(NTC):
                    ts = slice(TC * tc_i, TC * (tc_i + 1))
                    ps = pproj.tile([128, TC], f32, tag="pp")
                    for d in range(ND):
                        nc.tensor.matmul(
                            ps[:], w_sb[:, FPC * d:FPC * (d + 1)],
                            xtiles[d][:, ts], start=(d == 0), stop=False)
                    nc.tensor.matmul(ps[:], b_sb[:], ones_sb[:, ts],
                                     start=False, stop=True)
                    nc.vector.tensor_mul(x_sb[:, ts], ps[:], rope_sb[:, ts])

            # v_ext: [128, NS*VW]; per s block: [v_h0 | ones | v_h1 | ones]
            v_sb = vxpool.tile([128, NS * VW], bf16, tag="vext")
            nc.gpsimd.memset(v_sb[:, DH::DH + 1], 1.0)  # ones columns
            for s in range(NS):
                ss = slice(128 * s, 128 * (s + 1))
                ps = pproj.tile([128, FPC], f32, tag="pp")
                for d in range(ND):
                    nc.tensor.matmul(
                        ps[:], vin[d][:, ss], wv_sb[:, FPC * d:FPC * (d + 1)],
                        start=(d == 0), stop=False)
                nc.tensor.matmul(ps[:], ones_sb[:, 0:128], bv_sb[:],
                                 start=False, stop=True)
                # write [128,2,64] -> v_sb cols {VW*s..+64, VW*s+65..+129}
                o = VW * s
                nc.vector.tensor_copy(
                    v_sb[:, o:o + VW].rearrange(
                        "p (h w) -> p h w", h=2)[:, :, 0:DH],
                    ps.rearrange("p (h w) -> p h w", h=2))

            # ---- attention + A2A re-partition -------------------------
            a2a_in = [dram.tile([T // 2, ROWS], bf16, tag=f"a2ai{i}",
                                name=f"a2a_in{i}") for i in range(2)]
            a2a_out = [dram.tile([T // 2, ROWS], bf16, tag=f"a2ao{i}",
                                 name=f"a2a_out{i}") for i in range(2)]

            for tc_i in range(NTC):
                ts = slice(TC * tc_i, TC * (tc_i + 1))
                aT = [atpool.tile([DH, TC], bf16, tag=f"aT{h}",
                                  name=f"aT{tc_i}_{h}") for h in range(HPC)]
                up = [pU.tile([DH + 1, TC], f32, tag="pu",
                              name=f"up{tc_i}_{h}") for h in range(HPC)]
                for s in range(NS):
                    ss = slice(128 * s, 128 * (s + 1))
                    sp = pS.tile([128, 2 * TC], f32, tag="ps")
                    nc.tensor.matmul(sp[:, 0:TC], kt_sb[0:DH, ss],
                                     qt_sb[0:DH, ts], start=True, stop=True)
                    nc.tensor.matmul(sp[:, TC:2 * TC], kt_sb[DH:128, ss],
                                     qt_sb[DH:128, ts], start=True, stop=True,
                                     tile_position=(DH, 0))
                    ex = expool.tile([128, 2 * TC], bf16, tag="ex")
                    nc.scalar.activation(ex[:], sp[:], EXP, scale=0.125)
                    for h in range(HPC):
                        o = VW * s + (DH + 1) * h
                        nc.tensor.matmul(
                            up[h][:], v_sb[:, o:o + DH + 1],
                            ex[:, TC * h:TC * (h + 1)],
                            start=(s == 0), stop=(s == NS - 1))
                # normalize: attnT_h = U[0:64] * bcast(1/U[64])
                for h in range(HPC):
                    r_sb = nrmpool.tile([1, TC], f32, tag="rsb")
                    nc.vector.tensor_copy(r_sb[:], up[h][DH:DH + 1, :])
                    nc.vector.reciprocal(r_sb[:], r_sb[:])
                    rbc = nrmpool.tile([DH, TC], f32, tag="rbc")
                    nc.gpsimd.partition_broadcast(rbc[:], r_sb[:])
                    nc.vector.tensor_mul(aT[h][:], up[h][0:DH, :], rbc[:])
                # ship to the A2A bounce buffer: block j gets cols
                # [ROWS*j, ROWS*(j+1)) of this chunk, heads stacked
                half = tc_i // 2
                for j in range(TC // ROWS):  # 4 blocks per chunk
                    col = TC * (tc_i % 2) + ROWS * j
                    blk = (col // ROWS) * 128
                    cs = slice(ROWS * j, ROWS * (j + 1))
                    for h in range(HPC):
                        nc.sync.dma_start(
                            out=a2a_in[half][blk + DH * h:blk + DH * (h + 1), :],
                            in_=aT[h][:, cs])
                if tc_i % 2 == 1:
                    nc.gpsimd.collective_compute(
                        "AllToAll", mybir.AluOpType.bypass,
                        replica_groups=[list(range(N_CORES))],
                        ins=[a2a_in[half][:].opt()],
                        outs=[a2a_out[half][:].opt()],
                    )

            # ---- output projection on own 2 x ROWS rows ----------------
            wo_sb = win.tile([128, ND * D], bf16, tag="wo")
            nc.sync.dma_start(
                out=wo_sb[:].rearrange("p (d m) -> p d m", d=ND),
                in_=wo.rearrange("(d p) m -> p d m", p=128))
            for half, outx in ((0, out1), (1, out2)):
                ap = oprpool.tile([128, ND * ROWS], bf16, tag="aprj")
                nc.sync.dma_start(
                    out=ap[:].rearrange("p (d t) -> p d t", d=ND),
                    in_=a2a_out[half].rearrange("(d p) t -> p d t", p=128))
                oev = oprpool.tile([ROWS, D], f32, tag="oev")
                for n in range(2):
                    nsl = slice(512 * n, 512 * (n + 1))
                    po = pproj.tile([ROWS, 512], f32, tag="pp")
                    for d in range(ND):
                        nc.tensor.matmul(
                            po[:], ap[:, ROWS * d:ROWS * (d + 1)],
                            wo_sb[:, D * d + 512 * n:D * d + 512 * (n + 1)],
                            start=(d == 0), stop=False)
                    nc.tensor.matmul(po[:], ones_sb[:, 0:ROWS], bo_sb[:, nsl],
                                     start=False, stop=True)
                    nc.vector.tensor_copy(oev[:, nsl], po[:])
                nc.sync.dma_start(out=outx, in_=oev[:])

    nc.compile()
    return nc


def _host_inputs(query, key, value, Wq, bq, Wk, bk, Wv, bv, Wo, bo):
    """Shard + lay out the full inputs for the 8 cores."""
    b = ml_dtypes.bfloat16
    qT = np.ascontiguousarray(query.T).astype(b)
    kT = np.ascontiguousarray(key.T).astype(b)
    vT = np.ascontiguousarray(value.T).astype(b)
    wo = Wo.astype(b)

    theta = 1.0 / (ROPE_BASE ** (np.arange(0, D, 2, dtype=np.float32) / D))
    idx = np.outer(np.arange(T, dtype=np.float32), theta)
    c, s = np.cos(idx), np.sin(idx)
    C = np.concatenate([c + s, c - s], axis=1).astype(np.float32)  # [T, D]

    in_maps = []
    for cidx in range(N_CORES):
        fs = slice(FPC * cidx, FPC * (cidx + 1))
        in_maps.append({
            "qT": qT, "kT": kT, "vT": vT,
            "wq": Wq[:, fs].astype(b), "wk": Wk[:, fs].astype(b),
            "wv": Wv[:, fs].astype(b), "wo": wo,
            "bq": bq[None, fs].astype(b), "bk": bk[None, fs].astype(b),
            "bv": bv[None, fs].astype(b), "bo": bo[None, :].astype(b),
            "ropeC": np.ascontiguousarray(C[:, fs].T),
        })
    return in_maps


def kernel(query, key, value, Wq, bq, Wk, bk, Wv, bv, Wo, bo, _trace=False):
    if "nc" not in _cache:
        _cache["nc"] = _build()
    nc = _cache["nc"]
    in_maps = _host_inputs(query, key, value, Wq, bq, Wk, bk, Wv, bv, Wo, bo)
    res = run_bass_kernel_spmd(nc, in_maps, core_ids=list(range(N_CORES)),
                               trace=_trace)
    _cache["last_result"] = res
    out = np.empty((T, D), np.float32)
    for c in range(N_CORES):
        out[ROWS * c:ROWS * (c + 1), :] = res.results[c]["out1"]
        out[T // 2 + ROWS * c:T // 2 + ROWS * (c + 1), :] = \
            res.results[c]["out2"]
    return out
